# revision 12
# baseline (speedup 1.0000x reference)
"""Trainium2 Bass kernel for nn_EnoughViT_63282048139394.

Key mathematical reduction (verified exactly against the reference):
  - Attention in this architecture mixes ONLY the batch dimension, per
    sequence position ("scores = einsum('sbe,sce->sbc')").  No operation
    mixes sequence positions.
  - The classifier reads ONLY the last position (the class token), and
    that position's initial value (class_token + pos[:, -1]) is identical
    for every batch element, so it stays identical through every layer
    (mean-over-batch of identical rows is the row; the score matrix is a
    constant; LN/MLP act per-token).
  - Therefore the full [64, 1000] output is 64 identical copies of a
    single-token forward pass which does not depend on `x` at all:
        u = class_token + pos[-1]
        for l in 12:  h  = LN1(u); a = h@Wv; sval = h.(h@Wtheta)
                      u  = h + a*(1 + sval/sqrt(E))
                      h2 = LN2(u); u = u + gelu(h2@W1+b1)@W2 + b2
        out = log_softmax(gelu(LN_f(u)@Wc1+bc1)@Wc2 + bc2)  broadcast to 64

The kernel streams the ~305MB of weights from HBM through SBUF and runs
the GEMV chain on the tensor engine (token stationary, weights moving).
"""

import numpy as np
from contextlib import ExitStack

import concourse.bass as bass
import concourse.tile as tile
from concourse import bacc, mybir
from concourse.bass_utils import run_bass_kernel_spmd
from concourse.masks import make_identity

E = 768
HID = 3072
CLS = 1000
L = 12
EPS = 1e-5
INV_SQRT_E = 1.0 / float(np.sqrt(768.0))
DT = mybir.dt.float32
AX = mybir.AxisListType
OP = mybir.AluOpType
ACT = mybir.ActivationFunctionType

# n-tile split of a 768-wide GEMV output (<=512 each for fp32 moving operand)
N768 = (512, 256)
N1000 = (500, 500)


def build_program(gelu_mode='hw'):
    nc = bacc.Bacc()

    inp = {}

    def din(name, shape):
        t = nc.dram_tensor(name, list(shape), DT, kind="ExternalInput")
        inp[name] = t
        return t

    for l in range(L):
        din(f"wv{l}", (128, 6 * E))        # [p, s*768+n] = Wv[128s+p, n]
        din(f"wt{l}", (128, 6 * E))
        din(f"w1a{l}", (128, 3 * HID))     # s in 0..2
        din(f"w1b{l}", (128, 3 * HID))     # s in 3..5
        din(f"w2a{l}", (128, 12 * E))      # s in 0..11
        din(f"w2b{l}", (128, 12 * E))      # s in 12..23
        din(f"vec{l}", (1, 5 * E))         # ln1_s, ln1_b, ln2_s, ln2_b, b2
        din(f"b1cm{l}", (128, 24))         # b1 in cm layout [p,s]=b1[128s+p]
    din("wc1a", (128, 3 * HID))
    din("wc1b", (128, 3 * HID))
    din("wc2a", (128, 8 * CLS))            # s in 0..7
    din("wc2b", (128, 8 * CLS))            # s in 8..15
    din("wc2c", (128, 8 * CLS))            # s in 16..23
    din("fvec", (1, 2 * E + CLS))          # lnf_s, lnf_b, bc2
    din("bc1cm", (128, 24))
    din("u0", (1, E))

    out_t = nc.dram_tensor("out", [1, CLS], DT, kind="ExternalOutput")

    with ExitStack() as ctx:
        tc = ctx.enter_context(tile.TileContext(nc))
        wsm = ctx.enter_context(tc.tile_pool(name="wsm", bufs=1))
        wbg = ctx.enter_context(tc.tile_pool(name="wbg", bufs=2))
        vp = ctx.enter_context(tc.tile_pool(name="vp", bufs=1))
        pers = ctx.enter_context(tc.tile_pool(name="pers", bufs=1))
        wk = ctx.enter_context(tc.tile_pool(name="wk", bufs=1))
        ps_at = ctx.enter_context(tc.tile_pool(name="ps_at", bufs=1, space="PSUM"))
        ps_m = ctx.enter_context(tc.tile_pool(name="ps_m", bufs=2, space="PSUM"))
        ps_t = ctx.enter_context(tc.tile_pool(name="ps_t", bufs=1, space="PSUM"))

        ident = pers.tile([128, 128], DT)
        make_identity(nc, ident[:])
        epst = pers.tile([1, 1], DT)
        nc.vector.memset(epst[:], EPS)
        onet = pers.tile([1, 1], DT)
        nc.vector.memset(onet[:], 1.0)

        u = pers.tile([1, E], DT)
        nc.sync.dma_start(out=u[:], in_=inp["u0"][:, :])

        def gelu_inplace(x):
            """x <- gelu(x).  'hw': native ACT table. 'sim': tanh approx
            (CoreSim has no Gelu/Erf — used only for local plumbing checks)."""
            if gelu_mode == 'hw':
                nc.scalar.activation(out=x[:], in_=x[:], func=ACT.Gelu)
                return
            y = wk.tile(list(x.shape), DT, tag="geluy")
            nc.vector.tensor_mul(y[:], x[:], x[:])
            nc.vector.tensor_scalar(
                out=y[:], in0=y[:], scalar1=0.044715, scalar2=1.0,
                op0=OP.mult, op1=OP.add)
            nc.vector.tensor_mul(y[:], y[:], x[:])
            nc.scalar.activation(out=y[:], in_=y[:], func=ACT.Tanh,
                                 scale=float(np.sqrt(2.0 / np.pi)))
            nc.vector.tensor_scalar(
                out=y[:], in0=y[:], scalar1=1.0, scalar2=0.5,
                op0=OP.add, op1=OP.mult)
            nc.vector.tensor_mul(x[:], x[:], y[:])

        def layer_norm(x_ap, s_ap, b_ap, out_tile):
            """out = (x - mean(x)) * rsqrt(var(x)+EPS) * s + b   (flat [1,E'])"""
            n = x_ap.shape[-1]
            scr = wk.tile([1, n], DT, tag="lnscr")
            mean = wk.tile([1, 1], DT, tag="mean")
            msq = wk.tile([1, 1], DT, tag="msq")
            nc.vector.tensor_scalar(
                out=scr[:], in0=x_ap, scalar1=1.0 / n, scalar2=None,
                op0=OP.mult, op1=OP.add, accum_out=mean[:])
            nc.vector.tensor_mul(scr[:], x_ap, x_ap)
            nc.vector.tensor_scalar(
                out=scr[:], in0=scr[:], scalar1=1.0 / n, scalar2=None,
                op0=OP.mult, op1=OP.add, accum_out=msq[:])
            mu2 = wk.tile([1, 1], DT, tag="mu2")
            nc.vector.tensor_scalar(
                out=mu2[:], in0=mean[:], scalar1=mean[:], scalar2=None, op0=OP.mult)
            var = wk.tile([1, 1], DT, tag="var")
            nc.vector.tensor_sub(var[:], msq[:], mu2[:])
            sd = wk.tile([1, 1], DT, tag="sd")
            nc.scalar.activation(out=sd[:], in_=var[:], func=ACT.Sqrt, bias=epst[:])
            rstd = wk.tile([1, 1], DT, tag="rstd")
            nc.vector.reciprocal(rstd[:], sd[:])
            nc.vector.tensor_scalar(
                out=out_tile[:], in0=x_ap, scalar1=mean[:], scalar2=rstd[:],
                op0=OP.subtract, op1=OP.mult)
            nc.vector.tensor_mul(out_tile[:], out_tile[:], s_ap)
            nc.vector.tensor_add(out_tile[:], out_tile[:], b_ap)

        def to_cm(flat_tile, n_seg, tag):
            """[1, 128*n_seg] flat -> [128, n_seg] cm (cm[p,s]=flat[128s+p])."""
            ps = ps_t.tile([128, n_seg], DT, tag="tps")
            for s in range(n_seg):
                # out[p, s] = flat[128*s + p]: plain matmul, K=1, rhs=[[1.0]]
                nc.tensor.matmul(
                    ps[:, s:s + 1], flat_tile[0:1, 128 * s:128 * (s + 1)],
                    onet[:], start=True, stop=True)
            cm = wk.tile([128, n_seg], DT, tag=tag)
            nc.vector.tensor_copy(out=cm[:], in_=ps[:])
            return cm

        def gemv_768(lhs_cm, w_tile, n_s, tag):
            """[1,768] = token(lhs_cm [128,n_s]) @ W (w_tile rows 128s+p, 768 cols).
            Returns the two psum tiles (512, 256)."""
            outs = []
            for ni, (n0, nn) in enumerate(((0, 512), (512, 256))):
                pt = ps_at.tile([1, nn], DT, tag=f"{tag}{ni}")
                for s in range(n_s):
                    nc.tensor.matmul(
                        pt[:, :], lhs_cm[:, s:s + 1],
                        w_tile[:, s * E + n0: s * E + n0 + nn],
                        start=(s == 0), stop=(s == n_s - 1))
                outs.append(pt)
            return outs

        for l in range(L):
            vec = vp.tile([1, 5 * E], DT, tag="vec")
            nc.sync.dma_start(out=vec[:], in_=inp[f"vec{l}"][:, :])
            b1cm = vp.tile([128, 24], DT, tag="b1cm")
            nc.sync.dma_start(out=b1cm[:], in_=inp[f"b1cm{l}"][:, :])
            wv = wsm.tile([128, 6 * E], DT, tag="wv")
            nc.sync.dma_start(out=wv[:], in_=inp[f"wv{l}"][:, :])
            wt = wsm.tile([128, 6 * E], DT, tag="wt")
            nc.sync.dma_start(out=wt[:], in_=inp[f"wt{l}"][:, :])

            # ---- LN1 -> h ----
            h = wk.tile([1, E], DT, tag="h")
            layer_norm(u[:], vec[0:1, 0:E], vec[0:1, E:2 * E], h)
            hcm = to_cm(h, 6, "hcm")

            # ---- a = h@Wv, t = h@Wtheta ----
            a_ps = gemv_768(hcm, wv, 6, "a")
            t_ps = gemv_768(hcm, wt, 6, "t")

            tflat = wk.tile([1, E], DT, tag="tflat")
            nc.vector.tensor_copy(out=tflat[0:1, 0:512], in_=t_ps[0][:])
            nc.vector.tensor_copy(out=tflat[0:1, 512:768], in_=t_ps[1][:])

            # c0 = 1 + (h . t) / sqrt(E)
            scr = wk.tile([1, E], DT, tag="lnscr")
            sv = wk.tile([1, 1], DT, tag="sv")
            c0 = wk.tile([1, 1], DT, tag="c0")
            nc.vector.tensor_mul(scr[:], h[:], tflat[:])
            nc.vector.tensor_scalar(
                out=scr[:], in0=scr[:], scalar1=INV_SQRT_E, scalar2=None,
                op0=OP.mult, op1=OP.add, accum_out=sv[:])
            nc.vector.tensor_scalar(
                out=c0[:], in0=sv[:], scalar1=1.0, scalar2=None, op0=OP.add)

            # u = h + a * c0
            nc.vector.tensor_scalar(
                out=u[0:1, 0:512], in0=a_ps[0][:], scalar1=c0[:], scalar2=None,
                op0=OP.mult)
            nc.vector.tensor_scalar(
                out=u[0:1, 512:768], in0=a_ps[1][:], scalar1=c0[:], scalar2=None,
                op0=OP.mult)
            nc.vector.tensor_add(u[:], u[:], h[:])

            # ---- LN2 -> h2 ----
            h2 = wk.tile([1, E], DT, tag="h2")
            layer_norm(u[:], vec[0:1, 2 * E:3 * E], vec[0:1, 3 * E:4 * E], h2)
            h2cm = to_cm(h2, 6, "h2cm")

            # ---- m1 = h2@W1 (flat in sbuf), g = gelu(m1+b1) in cm ----
            w1a = wbg.tile([128, 3 * HID], DT, tag="wb")
            nc.sync.dma_start(out=w1a[:], in_=inp[f"w1a{l}"][:, :])
            w1b = wbg.tile([128, 3 * HID], DT, tag="wb")
            nc.sync.dma_start(out=w1b[:], in_=inp[f"w1b{l}"][:, :])
            gflat = wk.tile([1, HID], DT, tag="gflat")
            for nt in range(6):
                mp = ps_m.tile([1, 512], DT, tag="m")
                for s in range(6):
                    wsrc = w1a if s < 3 else w1b
                    sl = s % 3
                    nc.tensor.matmul(
                        mp[:, :], h2cm[:, s:s + 1],
                        wsrc[:, sl * HID + nt * 512: sl * HID + nt * 512 + 512],
                        start=(s == 0), stop=(s == 5))
                nc.vector.tensor_copy(
                    out=gflat[0:1, nt * 512:(nt + 1) * 512], in_=mp[:])
            gcm = to_cm(gflat, 24, "gcm")
            nc.vector.tensor_add(gcm[:], gcm[:], b1cm[:])
            gelu_inplace(gcm)

            # ---- m2 = g@W2 ; u = u + m2 + b2 ----
            w2a = wbg.tile([128, 12 * E], DT, tag="wb")
            nc.sync.dma_start(out=w2a[:], in_=inp[f"w2a{l}"][:, :])
            w2b = wbg.tile([128, 12 * E], DT, tag="wb")
            nc.sync.dma_start(out=w2b[:], in_=inp[f"w2b{l}"][:, :])
            for ni, (n0, nn) in enumerate(((0, 512), (512, 256))):
                mp2 = ps_m.tile([1, 512], DT, tag="m")
                for s in range(24):
                    wsrc = w2a if s < 12 else w2b
                    sl = s % 12
                    nc.tensor.matmul(
                        mp2[:, 0:nn], gcm[:, s:s + 1],
                        wsrc[:, sl * E + n0: sl * E + n0 + nn],
                        start=(s == 0), stop=(s == 23))
                nc.vector.tensor_add(
                    u[0:1, n0:n0 + nn], u[0:1, n0:n0 + nn], mp2[:, 0:nn])
            nc.vector.tensor_add(u[:], u[:], vec[0:1, 4 * E:5 * E])

        # ---- classifier ----
        fvec = vp.tile([1, 2 * E + CLS], DT, tag="vec")
        nc.sync.dma_start(out=fvec[:], in_=inp["fvec"][:, :])
        bc1cm = vp.tile([128, 24], DT, tag="b1cm")
        nc.sync.dma_start(out=bc1cm[:], in_=inp["bc1cm"][:, :])

        cf = wk.tile([1, E], DT, tag="h")
        layer_norm(u[:], fvec[0:1, 0:E], fvec[0:1, E:2 * E], cf)
        cfcm = to_cm(cf, 6, "hcm")

        wc1a = wbg.tile([128, 3 * HID], DT, tag="wb")
        nc.sync.dma_start(out=wc1a[:], in_=inp["wc1a"][:, :])
        wc1b = wbg.tile([128, 3 * HID], DT, tag="wb")
        nc.sync.dma_start(out=wc1b[:], in_=inp["wc1b"][:, :])
        g2flat = wk.tile([1, HID], DT, tag="gflat")
        for nt in range(6):
            mp = ps_m.tile([1, 512], DT, tag="m")
            for s in range(6):
                wsrc = wc1a if s < 3 else wc1b
                sl = s % 3
                nc.tensor.matmul(
                    mp[:, :], cfcm[:, s:s + 1],
                    wsrc[:, sl * HID + nt * 512: sl * HID + nt * 512 + 512],
                    start=(s == 0), stop=(s == 5))
            nc.vector.tensor_copy(
                out=g2flat[0:1, nt * 512:(nt + 1) * 512], in_=mp[:])
        g2cm = to_cm(g2flat, 24, "gcm")
        nc.vector.tensor_add(g2cm[:], g2cm[:], bc1cm[:])
        gelu_inplace(g2cm)

        wc2 = []
        for nm in ("wc2a", "wc2b", "wc2c"):
            w = wbg.tile([128, 8 * CLS], DT, tag="wb")
            nc.sync.dma_start(out=w[:], in_=inp[nm][:, :])
            wc2.append(w)
        lg = wk.tile([1, CLS], DT, tag="lg")
        for ni, (n0, nn) in enumerate(((0, 500), (500, 500))):
            lp = ps_m.tile([1, 500], DT, tag="m")
            for s in range(24):
                wsrc = wc2[s // 8]
                sl = s % 8
                nc.tensor.matmul(
                    lp[:, :], g2cm[:, s:s + 1],
                    wsrc[:, sl * CLS + n0: sl * CLS + n0 + nn],
                    start=(s == 0), stop=(s == 23))
            nc.vector.tensor_copy(out=lg[0:1, n0:n0 + nn], in_=lp[:])
        nc.vector.tensor_add(lg[:], lg[:], fvec[0:1, 2 * E:2 * E + CLS])

        # log_softmax
        mx = wk.tile([1, 1], DT, tag="mx")
        nc.vector.reduce_max(mx[:], lg[:], axis=AX.X)
        sh = wk.tile([1, CLS], DT, tag="sh")
        nc.vector.tensor_scalar(
            out=sh[:], in0=lg[:], scalar1=mx[:], scalar2=None, op0=OP.subtract)
        se = wk.tile([1, 1], DT, tag="se")
        nc.scalar.activation(out=lg[:], in_=sh[:], func=ACT.Exp, accum_out=se[:])
        lse = wk.tile([1, 1], DT, tag="lse")
        nc.scalar.activation(out=lse[:], in_=se[:], func=ACT.Ln)
        nc.vector.tensor_scalar(
            out=sh[:], in0=sh[:], scalar1=lse[:], scalar2=None, op0=OP.subtract)
        nc.sync.dma_start(out=out_t[:, :], in_=sh[:])

    nc.compile()
    return nc


def prep_inputs(inputs):
    """Numpy-side re-layout of the reference inputs into the DRAM tensors."""
    f32 = lambda x: np.ascontiguousarray(np.asarray(x, dtype=np.float32))
    m = {}
    Wv, Wt = inputs["Wv"], inputs["Wtheta"]
    W1, W2 = inputs["W1"], inputs["W2"]
    for l in range(L):
        # cm contraction layout: tile[p, s*N + n] = W[128s + p, n]
        m[f"wv{l}"] = f32(np.asarray(Wv[l]).reshape(6, 128, E).transpose(1, 0, 2).reshape(128, 6 * E))
        m[f"wt{l}"] = f32(np.asarray(Wt[l]).reshape(6, 128, E).transpose(1, 0, 2).reshape(128, 6 * E))
        w1 = np.asarray(W1[l]).reshape(6, 128, HID).transpose(1, 0, 2)
        m[f"w1a{l}"] = f32(w1[:, 0:3].reshape(128, 3 * HID))
        m[f"w1b{l}"] = f32(w1[:, 3:6].reshape(128, 3 * HID))
        w2 = np.asarray(W2[l]).reshape(24, 128, E).transpose(1, 0, 2)
        m[f"w2a{l}"] = f32(w2[:, 0:12].reshape(128, 12 * E))
        m[f"w2b{l}"] = f32(w2[:, 12:24].reshape(128, 12 * E))
        m[f"vec{l}"] = f32(np.concatenate([
            inputs["ln1_s"][l], inputs["ln1_b"][l],
            inputs["ln2_s"][l], inputs["ln2_b"][l],
            inputs["b2"][l]])).reshape(1, 5 * E)
        m[f"b1cm{l}"] = f32(np.asarray(inputs["b1"][l]).reshape(24, 128).T)
    wc1 = np.asarray(inputs["Wc1"]).reshape(6, 128, HID).transpose(1, 0, 2)
    m["wc1a"] = f32(wc1[:, 0:3].reshape(128, 3 * HID))
    m["wc1b"] = f32(wc1[:, 3:6].reshape(128, 3 * HID))
    wc2 = np.asarray(inputs["Wc2"]).reshape(24, 128, CLS).transpose(1, 0, 2)
    m["wc2a"] = f32(wc2[:, 0:8].reshape(128, 8 * CLS))
    m["wc2b"] = f32(wc2[:, 8:16].reshape(128, 8 * CLS))
    m["wc2c"] = f32(wc2[:, 16:24].reshape(128, 8 * CLS))
    m["fvec"] = f32(np.concatenate([
        inputs["lnf_s"], inputs["lnf_b"], inputs["bc2"]])).reshape(1, 2 * E + CLS)
    m["bc1cm"] = f32(np.asarray(inputs["bc1"]).reshape(24, 128).T)
    u0 = np.asarray(inputs["class_token"]).reshape(E) + np.asarray(inputs["pos"]).reshape(-1, E)[-1]
    m["u0"] = f32(u0).reshape(1, E)
    return m


_CACHED = {}


def kernel(**inputs) -> np.ndarray:
    b = int(np.asarray(inputs["x"]).shape[0])
    in_map = prep_inputs(inputs)
    if "nc" not in _CACHED:
        _CACHED["nc"] = build_program()
    nc = _CACHED["nc"]
    r = run_bass_kernel_spmd(nc, [in_map], core_ids=[0])
    out = np.asarray(r.results[0]["out"]).reshape(1, CLS)
    return np.ascontiguousarray(np.broadcast_to(out, (b, CLS)).astype(np.float32))


if __name__ == "__main__":
    import time
    d = np.load("/root/problem/inputs_cache.npz")
    inputs = {k: d[k] for k in d.files}
    t0 = time.time()
    out = kernel(**inputs)
    print("kernel wall time:", time.time() - t0)
    exp = np.load("/root/problem/expected.npy")
    err = np.abs(out - exp).max()
    rel = err / np.abs(exp).max()
    print("absmax err:", err, "rel:", rel)
